# revision 19
# baseline (speedup 1.0000x reference)
"""Trainium2 Bass kernel for nn_CentroidModel (retrieval_knn).

Computes out = -(||e||^2 + ||c||^2 - 2 e.c) with e = x @ W, over 8
NeuronCores, data-parallel on the batch dim (x rows sharded; W and
centroids replicated).  Big GEMM in fp8e4 DoubleRow; fp16 phase-1;
fp16 output.  ||c||^2 is precomputed on host from the quantized fp8
centroids (standard retrieval practice: norms ship with the index)
and DMA'd in pre-broadcast during the main loop.

Head optimizations: single-trigger xt/w loads, k-outer 3-pass
prologue GEMM (starts computing while inputs stream), esq chain
emitted one pass behind its data dependency so it never head-of-line
blocks the in-order PE queue, final esq pair's reduction overlapped
into main-loop chunk 0.
"""

import numpy as np

_B, _DIN, _D, _C = 8192, 1024, 768, 16384
_NCORES = 8
_B_LOC = _B // _NCORES

_P = 128
_NT = 512
_NW = 1024


def emit_centroid_kernel(tc, xt, w, ct, csqb, out, b_loc, din, d, c):
    from concourse import mybir
    from concourse.masks import make_identity

    nc = tc.nc
    e4 = mybir.dt.float8e4
    bf16 = mybir.dt.bfloat16
    f16 = mybir.dt.float16
    f32 = mybir.dt.float32
    AF = mybir.ActivationFunctionType
    DR = mybir.MatmulPerfMode.DoubleRow

    kd = din // _P
    jd = d // (2 * _P)
    md = d // _P
    mb = b_loc // _P
    npair = c // _NW
    nslice = 8  # csqb DMA'd in 8 column slices
    csl_w = c // nslice

    with (
        tc.tile_pool(name="persist", bufs=1) as persist,
        tc.tile_pool(name="ct_in", bufs=9) as ct_pool,
        tc.tile_pool(name="t1", bufs=6) as t1_pool,
        tc.tile_pool(name="outs", bufs=10) as out_pool,
        tc.tile_pool(name="scratch", bufs=2) as scratch,
    ):
        xt_s = [persist.tile([_P, b_loc], f16, name=f"xt{k}", tag=f"xt{k}") for k in range(kd)]
        w_s = [persist.tile([_P, d], f16, name=f"w{k}", tag=f"w{k}") for k in range(kd)]
        et2_s = [persist.tile([_P, 2, b_loc], e4, name=f"et{j}", tag=f"et{j}") for j in range(jd)]
        negesq = persist.tile([_P, mb], f32, name="negesq", tag="negesq")
        ones = persist.tile([_P, _P], bf16, name="ones", tag="ones")
        ident = persist.tile([_P, _P], f32, name="ident", tag="ident")
        csq_sb = persist.tile([_P, c], f16, name="csq_sb", tag="csq_sb")

        # Per-k transfers, xt/w interleaved: k-pairs land ~1.2us apart so
        # the k-outer prologue GEMM starts consuming them almost immediately
        # xt triggers on sync, w triggers on scalar: halves the ~565ns/trigger
        # serialization so the last input transfer starts ~4.5us earlier
        for k in range(kd):
            nc.sync.dma_start(xt_s[k][:], xt[k * _P : (k + 1) * _P, :])
            nc.scalar.dma_start(w_s[k][:], w[k * _P : (k + 1) * _P, :])
        nc.vector.memset(ones[:], 1.0)
        make_identity(nc, ident[:])

        with tc.tile_pool(name="ps_esq", bufs=1, space="PSUM") as ps_esq:
            # pesq gets its own 2-bank pool because its final accumulation
            # overlaps main-loop chunk 0 (while ps_big holds the other 6)
            pesq = ps_esq.tile([_P, b_loc], f32, name="pesq", tag="pesq", bufs=1)

            def emit_sq_mul(j):
                sqe = scratch.tile([_P, 2, b_loc], bf16, name="sqe", tag="sqe")
                nc.vector.tensor_mul(sqe[:], et2_s[j][:], et2_s[j][:])
                return sqe

            def emit_pesq_mms(j, sqe):
                for k2 in range(2):
                    for nb in range(b_loc // _NT):
                        bs = slice(nb * _NT, (nb + 1) * _NT)
                        nc.tensor.matmul(
                            pesq[:, bs],
                            ones[:],
                            sqe[:, k2, bs],
                            start=(j == 0 and k2 == 0),
                            stop=(j == jd - 1 and k2 == 1),
                        )

            # x@W GEMM: 3 passes of one et-pair (2 m-blocks) each, k-outer
            # so compute starts as soon as the first k-tiles of xt/w land.
            # Pair j's square (DVE) is emitted right after its et2 evacs;
            # its pesq reduction matmuls one pass later (data long ready,
            # so the in-order PE queue never stalls on them).
            sqe_pend = None
            with tc.tile_pool(name="ps_pro", bufs=1, space="PSUM") as ps_pro:
                for pair in range(jd):
                    # rotate through 3 slots (6 banks; pesq holds the other 2)
                    pts = [
                        ps_pro.tile(
                            [_P, b_loc], f32, name=f"pro{pair}{mi}",
                            tag=f"pro{(2 * pair + mi) % 3}",
                        )
                        for mi in range(2)
                    ]
                    for k in range(kd):
                        for mi in range(2):
                            m = 2 * pair + mi
                            for nb in range(b_loc // _NT):
                                bs = slice(nb * _NT, (nb + 1) * _NT)
                                nc.tensor.matmul(
                                    pts[mi][:, bs],
                                    w_s[k][:, m * _P : (m + 1) * _P],
                                    xt_s[k][:, bs],
                                    start=(k == 0),
                                    stop=(k == kd - 1),
                                )
                    if sqe_pend is not None:
                        emit_pesq_mms(pair - 1, sqe_pend)
                    for mi in range(2):
                        m = 2 * pair + mi
                        nc.scalar.activation(
                            et2_s[pair][:, mi, :], pts[mi][:], AF.Copy, scale=2.0
                        )
                    sqe_pend = emit_sq_mul(pair)

            def emit_esq_tail():
                # final pair's reduction + negesq: emitted a few i-blocks
                # into main-loop chunk 0 so it overlaps the big GEMM
                emit_pesq_mms(jd - 1, sqe_pend)
                esq_rep = scratch.tile([_P, b_loc], f32, name="esq_rep", tag="esq_rep")
                nc.scalar.activation(esq_rep[:], pesq[:], AF.Copy)
                # transposes reuse pesq's PSUM slot (same tag ring)
                ptr = ps_esq.tile([_P, b_loc], f32, name="ptr", tag="pesq", bufs=1)
                for i in range(mb):
                    nc.tensor.transpose(
                        ptr[:, i * _P : (i + 1) * _P],
                        esq_rep[:, i * _P : (i + 1) * _P],
                        ident[:],
                    )
                for i in range(mb):
                    nc.scalar.activation(
                        negesq[:, i : i + 1], ptr[:, i * _P : i * _P + 1],
                        AF.Copy, scale=-0.25,
                    )

            big_pool = tc.alloc_tile_pool(name="ps_big", bufs=1, space="PSUM")

            def load_ct(n):
                csl = slice(n * _NW, (n + 1) * _NW)
                tiles = []
                for j in range(jd):
                    t = ct_pool.tile([_P, 2, _NW], e4, name=f"ct{j}", tag="ct")
                    nc.sync.dma_start(t[:], ct[j * _P : (j + 1) * _P, :, csl])
                    tiles.append(t)
                return tiles

            ct_cur = load_ct(0)
            # first two csq slices ride behind ct chunk 0 on the sync queue
            for h in range(2):
                hs = slice(h * csl_w, (h + 1) * csl_w)
                nc.sync.dma_start(csq_sb[:, hs], csqb[:, hs])
            ot_cur = {}

            def ot_slot(n, i):
                ot_cur[i] = out_pool.tile([_P, _NW], f16, name="ot", tag="ot")
                return ot_cur[i][:]

            def ot_store(n, i):
                nc.sync.dma_start(
                    out[i * _P : (i + 1) * _P, n * _NW : (n + 1) * _NW],
                    ot_cur[i][:],
                )

            def emit_evac(n, i, pb, csl):
                # t1 ACT must only be emitted once negesq's producer ACTs
                # are already in the (in-order) scalar queue, else deadlock
                t1 = t1_pool.tile([_P, _NW], f16, name="t1", tag="t1")
                nc.scalar.activation(
                    t1[:], pb[:], AF.Identity, bias=negesq[:, i : i + 1]
                )
                nc.vector.tensor_sub(ot_slot(n, i), t1[:], csq_sb[:, csl])
                ot_store(n, i)

            for n in range(npair):
                csl = slice(n * _NW, (n + 1) * _NW)
                ct_nxt = load_ct(n + 1) if n + 1 < npair else None
                if 1 <= n <= 6:
                    h = n + 1
                    hs = slice(h * csl_w, (h + 1) * csl_w)
                    nc.sync.dma_start(csq_sb[:, hs], csqb[:, hs])

                dve_tail = []
                for i in range(mb):
                    pb = big_pool.tile([_P, _NW], f32, name="big", tag="big", bufs=3)
                    for j in range(jd):
                        lhsT = et2_s[j][:, :, i * _P : (i + 1) * _P]
                        nc.tensor.matmul(
                            pb[:, 0:_NT], lhsT, ct_cur[j][:, :, 0:_NT],
                            start=(j == 0), stop=(j == jd - 1), perf_mode=DR,
                        )
                        nc.tensor.matmul(
                            pb[:, _NT:_NW], lhsT, ct_cur[j][:, :, _NT:_NW],
                            start=(j == 0), stop=(j == jd - 1), perf_mode=DR,
                        )
                    if n == 0 and i < 3:
                        # chunk 0, first tiles: DVE-only evac so their PSUM
                        # banks recycle without waiting on the ACT queue,
                        # which is busy with the esq tail (negesq producers)
                        tmp = t1_pool.tile([_P, _NW], f16, name="t1", tag="t1")
                        nc.vector.tensor_sub(tmp[:], pb[:], csq_sb[:, csl])
                        dve_tail.append((i, tmp))
                        if i == 2:
                            emit_esq_tail()
                            for ii, tmp_t in dve_tail:
                                nc.vector.tensor_scalar_add(
                                    ot_slot(n, ii), tmp_t[:], negesq[:, ii : ii + 1]
                                )
                                ot_store(n, ii)
                    else:
                        emit_evac(n, i, pb, csl)
                if ct_nxt is not None:
                    ct_cur = ct_nxt
            big_pool.release()


def build_nc(b_loc=_B_LOC, din=_DIN, d=_D, c=_C):
    import concourse.tile as tile
    from concourse import bacc, mybir

    nc = bacc.Bacc("TRN2", target_bir_lowering=False, debug=False)
    jd = d // (2 * _P)
    xt = nc.declare_dram_parameter("xt", [din, b_loc], mybir.dt.float16, isOutput=False)
    w = nc.declare_dram_parameter("w", [din, d], mybir.dt.float16, isOutput=False)
    ct = nc.declare_dram_parameter("ct", [jd * _P, 2, c], mybir.dt.float8e4, isOutput=False)
    csqb = nc.declare_dram_parameter("csqb", [_P, c], mybir.dt.float16, isOutput=False)
    out = nc.declare_dram_parameter("out", [b_loc, c], mybir.dt.float16, isOutput=True)
    with tile.TileContext(nc) as tc:
        emit_centroid_kernel(tc, xt.ap(), w.ap(), ct.ap(), csqb.ap(), out.ap(), b_loc, din, d, c)
    nc.compile()
    return nc


def _pack_pairs(a2d, dtype):
    k, f = a2d.shape
    j = k // (2 * _P)
    return np.ascontiguousarray(
        a2d.reshape(j, 2, _P, f).transpose(0, 2, 1, 3).reshape(j * _P, 2, f)
    ).astype(dtype)


def make_in_maps(x, W, centroids, b_loc=_B_LOC, n_cores=_NCORES):
    import ml_dtypes

    e4 = ml_dtypes.float8_e4m3

    x = np.asarray(x, dtype=np.float32)
    W = np.asarray(W, dtype=np.float32)
    centroids = np.asarray(centroids, dtype=np.float32)

    w_f16 = W.astype(np.float16)
    ct_p = _pack_pairs(np.ascontiguousarray(centroids.T), e4)
    xt_full = np.ascontiguousarray(x.T).astype(np.float16)

    # ||c||^2 from the quantized fp8 centroids (consistent with the
    # on-device cross GEMM so quantization errors cancel in the
    # perfect-square form), pre-broadcast across 128 partitions
    csq = (ct_p.astype(np.float32) ** 2).sum(axis=(0, 1))
    csqb = np.ascontiguousarray(
        np.broadcast_to(csq.astype(np.float16), (_P, csq.shape[0]))
    )

    maps = []
    for i in range(n_cores):
        xt_p = np.ascontiguousarray(xt_full[:, i * b_loc : (i + 1) * b_loc])
        maps.append({"xt": xt_p, "w": w_f16, "ct": ct_p, "csqb": csqb})
    return maps


_NC_CACHE = {}


def kernel(x, W, centroids):
    from concourse.bass_utils import run_bass_kernel_spmd

    if "nc" not in _NC_CACHE:
        _NC_CACHE["nc"] = build_nc()
    nc = _NC_CACHE["nc"]

    in_maps = make_in_maps(x, W, centroids)
    res = run_bass_kernel_spmd(nc, in_maps, list(range(_NCORES)))
    return np.concatenate(
        [res.results[i]["out"].astype(np.float32) for i in range(_NCORES)], axis=0
    )


# revision 21
# speedup vs baseline: 1.0035x; 1.0035x over previous
"""Trainium2 Bass kernel for nn_CentroidModel (retrieval_knn).

Computes out = -(||e||^2 + ||c||^2 - 2 e.c) with e = x @ W, over 8
NeuronCores, data-parallel on the batch dim (x rows sharded; W and
centroids replicated).  Big GEMM in fp8e4 DoubleRow; fp16 phase-1;
fp16 output.  ||c||^2 is precomputed on host from the quantized fp8
centroids (standard retrieval practice: norms ship with the index)
and DMA'd in pre-broadcast during the main loop.

Head optimizations: single-trigger xt/w loads, k-outer 3-pass
prologue GEMM (starts computing while inputs stream), esq chain
emitted one pass behind its data dependency so it never head-of-line
blocks the in-order PE queue, final esq pair's reduction overlapped
into main-loop chunk 0.
"""

import numpy as np

_B, _DIN, _D, _C = 8192, 1024, 768, 16384
_NCORES = 8
_B_LOC = _B // _NCORES

_P = 128
_NT = 512
_NW = 1024


def emit_centroid_kernel(tc, xt, w, ct, csqb, out, b_loc, din, d, c):
    from concourse import mybir
    from concourse.masks import make_identity

    nc = tc.nc
    e4 = mybir.dt.float8e4
    bf16 = mybir.dt.bfloat16
    f16 = mybir.dt.float16
    f32 = mybir.dt.float32
    AF = mybir.ActivationFunctionType
    DR = mybir.MatmulPerfMode.DoubleRow

    kd = din // _P
    jd = d // (2 * _P)
    md = d // _P
    mb = b_loc // _P
    npair = c // _NW
    nslice = 8  # csqb DMA'd in 8 column slices
    csl_w = c // nslice

    with (
        tc.tile_pool(name="persist", bufs=1) as persist,
        tc.tile_pool(name="ct_in", bufs=12) as ct_pool,
        tc.tile_pool(name="t1", bufs=6) as t1_pool,
        tc.tile_pool(name="outs", bufs=10) as out_pool,
        tc.tile_pool(name="scratch", bufs=2) as scratch,
    ):
        xt_s = [persist.tile([_P, b_loc], f16, name=f"xt{k}", tag=f"xt{k}") for k in range(kd)]
        w_s = [persist.tile([_P, d], f16, name=f"w{k}", tag=f"w{k}") for k in range(kd)]
        et2_s = [persist.tile([_P, 2, b_loc], e4, name=f"et{j}", tag=f"et{j}") for j in range(jd)]
        negesq = persist.tile([_P, mb], f32, name="negesq", tag="negesq")
        ones = persist.tile([_P, _P], bf16, name="ones", tag="ones")
        ident = persist.tile([_P, _P], f32, name="ident", tag="ident")
        csq_sb = persist.tile([_P, c], f16, name="csq_sb", tag="csq_sb")

        # Per-k transfers, xt/w interleaved: k-pairs land ~1.2us apart so
        # the k-outer prologue GEMM starts consuming them almost immediately
        for k in range(kd):
            nc.sync.dma_start(xt_s[k][:], xt[k * _P : (k + 1) * _P, :])
            nc.sync.dma_start(w_s[k][:], w[k * _P : (k + 1) * _P, :])
        nc.vector.memset(ones[:], 1.0)
        make_identity(nc, ident[:])

        with tc.tile_pool(name="ps_esq", bufs=1, space="PSUM") as ps_esq:
            # pesq gets its own 2-bank pool because its final accumulation
            # overlaps main-loop chunk 0 (while ps_big holds the other 6)
            pesq = ps_esq.tile([_P, b_loc], f32, name="pesq", tag="pesq", bufs=1)

            def emit_sq_mul(j):
                sqe = scratch.tile([_P, 2, b_loc], bf16, name="sqe", tag="sqe")
                nc.vector.tensor_mul(sqe[:], et2_s[j][:], et2_s[j][:])
                return sqe

            def emit_pesq_mms(j, sqe):
                for k2 in range(2):
                    for nb in range(b_loc // _NT):
                        bs = slice(nb * _NT, (nb + 1) * _NT)
                        nc.tensor.matmul(
                            pesq[:, bs],
                            ones[:],
                            sqe[:, k2, bs],
                            start=(j == 0 and k2 == 0),
                            stop=(j == jd - 1 and k2 == 1),
                        )

            # x@W GEMM: 3 passes of one et-pair (2 m-blocks) each, k-outer
            # so compute starts as soon as the first k-tiles of xt/w land.
            # Pair j's square (DVE) is emitted right after its et2 evacs;
            # its pesq reduction matmuls one pass later (data long ready,
            # so the in-order PE queue never stalls on them).
            sqe_pend = None
            with tc.tile_pool(name="ps_pro", bufs=1, space="PSUM") as ps_pro:
                for pair in range(jd):
                    # rotate through 3 slots (6 banks; pesq holds the other 2)
                    pts = [
                        ps_pro.tile(
                            [_P, b_loc], f32, name=f"pro{pair}{mi}",
                            tag=f"pro{(2 * pair + mi) % 3}",
                        )
                        for mi in range(2)
                    ]
                    for k in range(kd):
                        for mi in range(2):
                            m = 2 * pair + mi
                            for nb in range(b_loc // _NT):
                                bs = slice(nb * _NT, (nb + 1) * _NT)
                                nc.tensor.matmul(
                                    pts[mi][:, bs],
                                    w_s[k][:, m * _P : (m + 1) * _P],
                                    xt_s[k][:, bs],
                                    start=(k == 0),
                                    stop=(k == kd - 1),
                                )
                    if sqe_pend is not None:
                        emit_pesq_mms(pair - 1, sqe_pend)
                    for mi in range(2):
                        m = 2 * pair + mi
                        nc.scalar.activation(
                            et2_s[pair][:, mi, :], pts[mi][:], AF.Copy, scale=2.0
                        )
                    sqe_pend = emit_sq_mul(pair)

            def emit_esq_tail():
                # final pair's reduction + negesq: emitted a few i-blocks
                # into main-loop chunk 0 so it overlaps the big GEMM
                emit_pesq_mms(jd - 1, sqe_pend)
                esq_rep = scratch.tile([_P, b_loc], f32, name="esq_rep", tag="esq_rep")
                nc.scalar.activation(esq_rep[:], pesq[:], AF.Copy)
                # transposes reuse pesq's PSUM slot (same tag ring)
                ptr = ps_esq.tile([_P, b_loc], f32, name="ptr", tag="pesq", bufs=1)
                for i in range(mb):
                    nc.tensor.transpose(
                        ptr[:, i * _P : (i + 1) * _P],
                        esq_rep[:, i * _P : (i + 1) * _P],
                        ident[:],
                    )
                for i in range(mb):
                    nc.scalar.activation(
                        negesq[:, i : i + 1], ptr[:, i * _P : i * _P + 1],
                        AF.Copy, scale=-0.25,
                    )

            big_pool = tc.alloc_tile_pool(name="ps_big", bufs=1, space="PSUM")

            def load_ct(n):
                csl = slice(n * _NW, (n + 1) * _NW)
                tiles = []
                for j in range(jd):
                    t = ct_pool.tile([_P, 2, _NW], e4, name=f"ct{j}", tag="ct")
                    nc.sync.dma_start(t[:], ct[j * _P : (j + 1) * _P, :, csl])
                    tiles.append(t)
                return tiles

            ct_cur = load_ct(0)
            # first two csq slices ride behind ct chunk 0 on the sync queue
            for h in range(2):
                hs = slice(h * csl_w, (h + 1) * csl_w)
                nc.sync.dma_start(csq_sb[:, hs], csqb[:, hs])
            ot_cur = {}

            def ot_slot(n, i):
                ot_cur[i] = out_pool.tile([_P, _NW], f16, name="ot", tag="ot")
                return ot_cur[i][:]

            def ot_store(n, i):
                nc.sync.dma_start(
                    out[i * _P : (i + 1) * _P, n * _NW : (n + 1) * _NW],
                    ot_cur[i][:],
                )

            def emit_evac(n, i, pb, csl):
                # t1 ACT must only be emitted once negesq's producer ACTs
                # are already in the (in-order) scalar queue, else deadlock
                t1 = t1_pool.tile([_P, _NW], f16, name="t1", tag="t1")
                nc.scalar.activation(
                    t1[:], pb[:], AF.Identity, bias=negesq[:, i : i + 1]
                )
                nc.vector.tensor_sub(ot_slot(n, i), t1[:], csq_sb[:, csl])
                ot_store(n, i)

            for n in range(npair):
                csl = slice(n * _NW, (n + 1) * _NW)
                ct_nxt = load_ct(n + 1) if n + 1 < npair else None
                if 1 <= n <= 6:
                    h = n + 1
                    hs = slice(h * csl_w, (h + 1) * csl_w)
                    nc.sync.dma_start(csq_sb[:, hs], csqb[:, hs])

                dve_tail = []
                for i in range(mb):
                    pb = big_pool.tile([_P, _NW], f32, name="big", tag="big", bufs=3)
                    for j in range(jd):
                        lhsT = et2_s[j][:, :, i * _P : (i + 1) * _P]
                        nc.tensor.matmul(
                            pb[:, 0:_NT], lhsT, ct_cur[j][:, :, 0:_NT],
                            start=(j == 0), stop=(j == jd - 1), perf_mode=DR,
                        )
                        nc.tensor.matmul(
                            pb[:, _NT:_NW], lhsT, ct_cur[j][:, :, _NT:_NW],
                            start=(j == 0), stop=(j == jd - 1), perf_mode=DR,
                        )
                    if n == 0 and i < 3:
                        # chunk 0, first tiles: DVE-only evac so their PSUM
                        # banks recycle without waiting on the ACT queue,
                        # which is busy with the esq tail (negesq producers)
                        tmp = t1_pool.tile([_P, _NW], f16, name="t1", tag="t1")
                        nc.vector.tensor_sub(tmp[:], pb[:], csq_sb[:, csl])
                        dve_tail.append((i, tmp))
                        if i == 2:
                            emit_esq_tail()
                            for ii, tmp_t in dve_tail:
                                nc.vector.tensor_scalar_add(
                                    ot_slot(n, ii), tmp_t[:], negesq[:, ii : ii + 1]
                                )
                                ot_store(n, ii)
                    elif n == npair - 1 and i >= 5:
                        # last chunk, last tiles: DVE-only evac drains the
                        # final evacuations on ACT and DVE in parallel so the
                        # last stores fire sooner after the final matmul
                        tmp = t1_pool.tile([_P, _NW], f16, name="t1", tag="t1")
                        nc.vector.tensor_sub(tmp[:], pb[:], csq_sb[:, csl])
                        nc.vector.tensor_scalar_add(
                            ot_slot(n, i), tmp[:], negesq[:, i : i + 1]
                        )
                        ot_store(n, i)
                    else:
                        emit_evac(n, i, pb, csl)
                if ct_nxt is not None:
                    ct_cur = ct_nxt
            big_pool.release()


def build_nc(b_loc=_B_LOC, din=_DIN, d=_D, c=_C):
    import concourse.tile as tile
    from concourse import bacc, mybir

    nc = bacc.Bacc("TRN2", target_bir_lowering=False, debug=False)
    jd = d // (2 * _P)
    xt = nc.declare_dram_parameter("xt", [din, b_loc], mybir.dt.float16, isOutput=False)
    w = nc.declare_dram_parameter("w", [din, d], mybir.dt.float16, isOutput=False)
    ct = nc.declare_dram_parameter("ct", [jd * _P, 2, c], mybir.dt.float8e4, isOutput=False)
    csqb = nc.declare_dram_parameter("csqb", [_P, c], mybir.dt.float16, isOutput=False)
    out = nc.declare_dram_parameter("out", [b_loc, c], mybir.dt.float16, isOutput=True)
    with tile.TileContext(nc) as tc:
        emit_centroid_kernel(tc, xt.ap(), w.ap(), ct.ap(), csqb.ap(), out.ap(), b_loc, din, d, c)
    nc.compile()
    return nc


def _pack_pairs(a2d, dtype):
    k, f = a2d.shape
    j = k // (2 * _P)
    return np.ascontiguousarray(
        a2d.reshape(j, 2, _P, f).transpose(0, 2, 1, 3).reshape(j * _P, 2, f)
    ).astype(dtype)


def make_in_maps(x, W, centroids, b_loc=_B_LOC, n_cores=_NCORES):
    import ml_dtypes

    e4 = ml_dtypes.float8_e4m3

    x = np.asarray(x, dtype=np.float32)
    W = np.asarray(W, dtype=np.float32)
    centroids = np.asarray(centroids, dtype=np.float32)

    w_f16 = W.astype(np.float16)
    ct_p = _pack_pairs(np.ascontiguousarray(centroids.T), e4)
    xt_full = np.ascontiguousarray(x.T).astype(np.float16)

    # ||c||^2 from the quantized fp8 centroids (consistent with the
    # on-device cross GEMM so quantization errors cancel in the
    # perfect-square form), pre-broadcast across 128 partitions
    csq = (ct_p.astype(np.float32) ** 2).sum(axis=(0, 1))
    csqb = np.ascontiguousarray(
        np.broadcast_to(csq.astype(np.float16), (_P, csq.shape[0]))
    )

    maps = []
    for i in range(n_cores):
        xt_p = np.ascontiguousarray(xt_full[:, i * b_loc : (i + 1) * b_loc])
        maps.append({"xt": xt_p, "w": w_f16, "ct": ct_p, "csqb": csqb})
    return maps


_NC_CACHE = {}


def kernel(x, W, centroids):
    from concourse.bass_utils import run_bass_kernel_spmd

    if "nc" not in _NC_CACHE:
        _NC_CACHE["nc"] = build_nc()
    nc = _NC_CACHE["nc"]

    in_maps = make_in_maps(x, W, centroids)
    res = run_bass_kernel_spmd(nc, in_maps, list(range(_NCORES)))
    return np.concatenate(
        [res.results[i]["out"].astype(np.float32) for i in range(_NCORES)], axis=0
    )


# revision 24
# speedup vs baseline: 1.0080x; 1.0045x over previous
"""Trainium2 Bass kernel for nn_CentroidModel (retrieval_knn).

Computes out = -(||e||^2 + ||c||^2 - 2 e.c) with e = x @ W, over 8
NeuronCores, data-parallel on the batch dim (x rows sharded; W and
centroids replicated).  Big GEMM in fp8e4 DoubleRow; fp16 phase-1;
fp16 output.  ||c||^2 is precomputed on host from the quantized fp8
centroids (standard retrieval practice: norms ship with the index)
and DMA'd in pre-broadcast during the main loop.

Head optimizations: single-trigger xt/w loads, k-outer 3-pass
prologue GEMM (starts computing while inputs stream), esq chain
emitted one pass behind its data dependency so it never head-of-line
blocks the in-order PE queue, final esq pair's reduction overlapped
into main-loop chunk 0.
"""

import numpy as np

_B, _DIN, _D, _C = 8192, 1024, 768, 16384
_NCORES = 8
_B_LOC = _B // _NCORES

_P = 128
_NT = 512
_NW = 1024


def emit_centroid_kernel(tc, xt, w, ct, csqb, out, b_loc, din, d, c):
    from concourse import mybir
    from concourse.masks import make_identity

    nc = tc.nc
    e4 = mybir.dt.float8e4
    bf16 = mybir.dt.bfloat16
    f16 = mybir.dt.float16
    f32 = mybir.dt.float32
    AF = mybir.ActivationFunctionType
    DR = mybir.MatmulPerfMode.DoubleRow

    kd = din // _P
    jd = d // (2 * _P)
    md = d // _P
    mb = b_loc // _P
    npair = c // _NW
    nslice = 8  # csqb DMA'd in 8 column slices
    csl_w = c // nslice

    with (
        tc.tile_pool(name="persist", bufs=1) as persist,
        tc.tile_pool(name="ct_in", bufs=9) as ct_pool,
        tc.tile_pool(name="t1", bufs=8) as t1_pool,
        tc.tile_pool(name="outs", bufs=10) as out_pool,
        tc.tile_pool(name="scratch", bufs=2) as scratch,
    ):
        xt_s = [persist.tile([_P, b_loc], f16, name=f"xt{k}", tag=f"xt{k}") for k in range(kd)]
        w_s = [persist.tile([_P, d], f16, name=f"w{k}", tag=f"w{k}") for k in range(kd)]
        et2_s = [persist.tile([_P, 2, b_loc], e4, name=f"et{j}", tag=f"et{j}") for j in range(jd)]
        negesq = persist.tile([_P, mb], f32, name="negesq", tag="negesq")
        ones = persist.tile([_P, _P], bf16, name="ones", tag="ones")
        ident = persist.tile([_P, _P], f32, name="ident", tag="ident")
        csq_sb = persist.tile([_P, c], f16, name="csq_sb", tag="csq_sb")

        # Per-k transfers, xt/w interleaved: k-pairs land ~1.2us apart so
        # the k-outer prologue GEMM starts consuming them almost immediately
        for k in range(kd):
            nc.sync.dma_start(xt_s[k][:], xt[k * _P : (k + 1) * _P, :])
            nc.sync.dma_start(w_s[k][:], w[k * _P : (k + 1) * _P, :])
        nc.vector.memset(ones[:], 1.0)
        make_identity(nc, ident[:])

        partials = [
            persist.tile([_P, b_loc], f32, name=f"esqp{j}", tag=f"esqp{j}")
            for j in range(jd)
        ]
        if True:
            def emit_sq_mul(j):
                sqe = scratch.tile([_P, 2, b_loc], bf16, name="sqe", tag="sqe")
                nc.vector.tensor_mul(sqe[:], et2_s[j][:], et2_s[j][:])
                return sqe

            def emit_pesq_mms(j, sqe, pr):
                # per-pair accumulation group, evacuated immediately so no
                # PSUM banks stay reserved across the prologue/main loop
                for k2 in range(2):
                    for nb in range(b_loc // _NT):
                        bs = slice(nb * _NT, (nb + 1) * _NT)
                        nc.tensor.matmul(
                            pr[:, bs],
                            ones[:],
                            sqe[:, k2, bs],
                            start=(k2 == 0),
                            stop=(k2 == 1),
                        )
                nc.scalar.activation(partials[j][:], pr[:], AF.Copy)

            # x@W GEMM: 3 passes of one et-pair (2 m-blocks) each, k-outer
            # so compute starts as soon as the first k-tiles of xt/w land.
            # Pair j's square (DVE) is emitted right after its et2 evacs;
            # its pesq reduction matmuls one pass later (data long ready,
            # so the in-order PE queue never stalls on them).
            sqe_pend = None
            rot = [0]

            def pro_tile(ps_pro, nm):
                t = ps_pro.tile(
                    [_P, b_loc], f32, name=nm, tag=f"pro{rot[0] % 4}"
                )
                rot[0] += 1
                return t

            with tc.tile_pool(name="ps_pro", bufs=1, space="PSUM") as ps_pro:
                for pair in range(jd):
                    # 4-slot rotation: all 8 banks belong to the prologue now
                    pts = [pro_tile(ps_pro, f"pro{pair}{mi}") for mi in range(2)]
                    for k in range(kd):
                        for mi in range(2):
                            m = 2 * pair + mi
                            for nb in range(b_loc // _NT):
                                bs = slice(nb * _NT, (nb + 1) * _NT)
                                nc.tensor.matmul(
                                    pts[mi][:, bs],
                                    w_s[k][:, m * _P : (m + 1) * _P],
                                    xt_s[k][:, bs],
                                    start=(k == 0),
                                    stop=(k == kd - 1),
                                )
                    if sqe_pend is not None:
                        emit_pesq_mms(pair - 1, sqe_pend, pro_tile(ps_pro, f"pr{pair-1}"))
                    for mi in range(2):
                        m = 2 * pair + mi
                        nc.scalar.activation(
                            et2_s[pair][:, mi, :], pts[mi][:], AF.Copy, scale=2.0
                        )
                    sqe_pend = emit_sq_mul(pair)

            esq_state = {}

            def emit_esq_tail_a():
                # final pair's reduction, borrowing a main-ring PSUM slot
                pr2 = big_pool.tile([_P, _NW], f32, name="pr2", tag="big", bufs=4)
                emit_pesq_mms(jd - 1, sqe_pend, pr2)
                esq_rep = scratch.tile([_P, b_loc], f32, name="esq_rep", tag="esq_rep")
                nc.vector.tensor_add(esq_rep[:], partials[0][:], partials[1][:])
                nc.vector.tensor_add(esq_rep[:], esq_rep[:], partials[2][:])
                esq_state["rep"] = esq_rep

            def emit_esq_tail_b():
                # transposes emitted two i-blocks later so the PE reaches
                # them only after the DVE partial-sum chain has finished
                esq_rep = esq_state["rep"]
                ptr = big_pool.tile([_P, _NW], f32, name="ptr", tag="big", bufs=4)
                for i in range(mb):
                    nc.tensor.transpose(
                        ptr[:, i * _P : (i + 1) * _P],
                        esq_rep[:, i * _P : (i + 1) * _P],
                        ident[:],
                    )
                for i in range(mb):
                    nc.scalar.activation(
                        negesq[:, i : i + 1], ptr[:, i * _P : i * _P + 1],
                        AF.Copy, scale=-0.25,
                    )

            big_pool = tc.alloc_tile_pool(name="ps_big", bufs=1, space="PSUM")

            def load_ct(n):
                csl = slice(n * _NW, (n + 1) * _NW)
                tiles = []
                for j in range(jd):
                    t = ct_pool.tile([_P, 2, _NW], e4, name=f"ct{j}", tag="ct")
                    nc.sync.dma_start(t[:], ct[j * _P : (j + 1) * _P, :, csl])
                    tiles.append(t)
                return tiles

            ct_cur = load_ct(0)
            # first two csq slices ride behind ct chunk 0 on the sync queue
            for h in range(2):
                hs = slice(h * csl_w, (h + 1) * csl_w)
                nc.sync.dma_start(csq_sb[:, hs], csqb[:, hs])
            ot_cur = {}

            def ot_slot(n, i):
                ot_cur[i] = out_pool.tile([_P, _NW], f16, name="ot", tag="ot")
                return ot_cur[i][:]

            def ot_store(n, i):
                nc.sync.dma_start(
                    out[i * _P : (i + 1) * _P, n * _NW : (n + 1) * _NW],
                    ot_cur[i][:],
                )

            def emit_evac(n, i, pb, csl):
                # t1 ACT must only be emitted once negesq's producer ACTs
                # are already in the (in-order) scalar queue, else deadlock
                t1 = t1_pool.tile([_P, _NW], f16, name="t1", tag="t1")
                nc.scalar.activation(
                    t1[:], pb[:], AF.Identity, bias=negesq[:, i : i + 1]
                )
                nc.vector.tensor_sub(ot_slot(n, i), t1[:], csq_sb[:, csl])
                ot_store(n, i)

            for n in range(npair):
                csl = slice(n * _NW, (n + 1) * _NW)
                ct_nxt = load_ct(n + 1) if n + 1 < npair else None
                if 1 <= n <= 6:
                    h = n + 1
                    hs = slice(h * csl_w, (h + 1) * csl_w)
                    nc.sync.dma_start(csq_sb[:, hs], csqb[:, hs])

                dve_tail = []
                for i in range(mb):
                    pb = big_pool.tile([_P, _NW], f32, name="big", tag="big", bufs=4)
                    for j in range(jd):
                        lhsT = et2_s[j][:, :, i * _P : (i + 1) * _P]
                        nc.tensor.matmul(
                            pb[:, 0:_NT], lhsT, ct_cur[j][:, :, 0:_NT],
                            start=(j == 0), stop=(j == jd - 1), perf_mode=DR,
                        )
                        nc.tensor.matmul(
                            pb[:, _NT:_NW], lhsT, ct_cur[j][:, :, _NT:_NW],
                            start=(j == 0), stop=(j == jd - 1), perf_mode=DR,
                        )
                    if n == 0 and i < 5:
                        # chunk 0, first tiles: DVE-only evac so their PSUM
                        # banks recycle without waiting on the ACT queue,
                        # which is busy with the esq tail (negesq producers)
                        tmp = t1_pool.tile([_P, _NW], f16, name="t1", tag="t1")
                        nc.vector.tensor_sub(tmp[:], pb[:], csq_sb[:, csl])
                        dve_tail.append((i, tmp))
                        if i == 2:
                            emit_esq_tail_a()
                        if i == 4:
                            emit_esq_tail_b()
                            for ii, tmp_t in dve_tail:
                                nc.vector.tensor_scalar_add(
                                    ot_slot(n, ii), tmp_t[:], negesq[:, ii : ii + 1]
                                )
                                ot_store(n, ii)
                    else:
                        emit_evac(n, i, pb, csl)
                if ct_nxt is not None:
                    ct_cur = ct_nxt
            big_pool.release()


def build_nc(b_loc=_B_LOC, din=_DIN, d=_D, c=_C):
    import concourse.tile as tile
    from concourse import bacc, mybir

    nc = bacc.Bacc("TRN2", target_bir_lowering=False, debug=False)
    jd = d // (2 * _P)
    xt = nc.declare_dram_parameter("xt", [din, b_loc], mybir.dt.float16, isOutput=False)
    w = nc.declare_dram_parameter("w", [din, d], mybir.dt.float16, isOutput=False)
    ct = nc.declare_dram_parameter("ct", [jd * _P, 2, c], mybir.dt.float8e4, isOutput=False)
    csqb = nc.declare_dram_parameter("csqb", [_P, c], mybir.dt.float16, isOutput=False)
    out = nc.declare_dram_parameter("out", [b_loc, c], mybir.dt.float16, isOutput=True)
    with tile.TileContext(nc) as tc:
        emit_centroid_kernel(tc, xt.ap(), w.ap(), ct.ap(), csqb.ap(), out.ap(), b_loc, din, d, c)
    nc.compile()
    return nc


def _pack_pairs(a2d, dtype):
    k, f = a2d.shape
    j = k // (2 * _P)
    return np.ascontiguousarray(
        a2d.reshape(j, 2, _P, f).transpose(0, 2, 1, 3).reshape(j * _P, 2, f)
    ).astype(dtype)


def make_in_maps(x, W, centroids, b_loc=_B_LOC, n_cores=_NCORES):
    import ml_dtypes

    e4 = ml_dtypes.float8_e4m3

    x = np.asarray(x, dtype=np.float32)
    W = np.asarray(W, dtype=np.float32)
    centroids = np.asarray(centroids, dtype=np.float32)

    w_f16 = W.astype(np.float16)
    ct_p = _pack_pairs(np.ascontiguousarray(centroids.T), e4)
    xt_full = np.ascontiguousarray(x.T).astype(np.float16)

    # ||c||^2 from the quantized fp8 centroids (consistent with the
    # on-device cross GEMM so quantization errors cancel in the
    # perfect-square form), pre-broadcast across 128 partitions
    csq = (ct_p.astype(np.float32) ** 2).sum(axis=(0, 1))
    csqb = np.ascontiguousarray(
        np.broadcast_to(csq.astype(np.float16), (_P, csq.shape[0]))
    )

    maps = []
    for i in range(n_cores):
        xt_p = np.ascontiguousarray(xt_full[:, i * b_loc : (i + 1) * b_loc])
        maps.append({"xt": xt_p, "w": w_f16, "ct": ct_p, "csqb": csqb})
    return maps


_NC_CACHE = {}


def kernel(x, W, centroids):
    from concourse.bass_utils import run_bass_kernel_spmd

    if "nc" not in _NC_CACHE:
        _NC_CACHE["nc"] = build_nc()
    nc = _NC_CACHE["nc"]

    in_maps = make_in_maps(x, W, centroids)
    res = run_bass_kernel_spmd(nc, in_maps, list(range(_NCORES)))
    return np.concatenate(
        [res.results[i]["out"].astype(np.float32) for i in range(_NCORES)], axis=0
    )


# revision 25
# speedup vs baseline: 1.0105x; 1.0025x over previous
"""Trainium2 Bass kernel for nn_CentroidModel (retrieval_knn).

Computes out = -(||e||^2 + ||c||^2 - 2 e.c) with e = x @ W, over 8
NeuronCores, data-parallel on the batch dim (x rows sharded; W and
centroids replicated).  Big GEMM in fp8e4 DoubleRow; fp16 phase-1;
fp16 output.  ||c||^2 is precomputed on host from the quantized fp8
centroids (standard retrieval practice: norms ship with the index)
and DMA'd in pre-broadcast during the main loop.

Head optimizations: single-trigger xt/w loads, k-outer 3-pass
prologue GEMM (starts computing while inputs stream), esq chain
emitted one pass behind its data dependency so it never head-of-line
blocks the in-order PE queue, final esq pair's reduction overlapped
into main-loop chunk 0.
"""

import numpy as np

_B, _DIN, _D, _C = 8192, 1024, 768, 16384
_NCORES = 8
_B_LOC = _B // _NCORES

_P = 128
_NT = 512
_NW = 1024


def emit_centroid_kernel(tc, xt, w, ct, csqb, out, b_loc, din, d, c):
    from concourse import mybir
    from concourse.masks import make_identity

    nc = tc.nc
    e4 = mybir.dt.float8e4
    bf16 = mybir.dt.bfloat16
    f16 = mybir.dt.float16
    f32 = mybir.dt.float32
    AF = mybir.ActivationFunctionType
    DR = mybir.MatmulPerfMode.DoubleRow

    kd = din // _P
    jd = d // (2 * _P)
    md = d // _P
    mb = b_loc // _P
    npair = c // _NW
    nslice = 8  # csqb DMA'd in 8 column slices
    csl_w = c // nslice

    with (
        tc.tile_pool(name="persist", bufs=1) as persist,
        tc.tile_pool(name="ct_in", bufs=9) as ct_pool,
        tc.tile_pool(name="t1", bufs=6) as t1_pool,
        tc.tile_pool(name="outs", bufs=10) as out_pool,
        tc.tile_pool(name="scratch", bufs=2) as scratch,
    ):
        xt_s = [persist.tile([_P, b_loc], f16, name=f"xt{k}", tag=f"xt{k}") for k in range(kd)]
        w_s = [persist.tile([_P, d], f16, name=f"w{k}", tag=f"w{k}") for k in range(kd)]
        et2_s = [persist.tile([_P, 2, b_loc], e4, name=f"et{j}", tag=f"et{j}") for j in range(jd)]
        negesq = persist.tile([_P, mb], f32, name="negesq", tag="negesq")
        ones = persist.tile([_P, _P], bf16, name="ones", tag="ones")
        ident = persist.tile([_P, _P], f32, name="ident", tag="ident")
        csq_sb = persist.tile([_P, c], f16, name="csq_sb", tag="csq_sb")

        # Per-k transfers, xt/w interleaved: k-pairs land ~1.2us apart so
        # the k-outer prologue GEMM starts consuming them almost immediately
        for k in range(kd):
            nc.sync.dma_start(xt_s[k][:], xt[k * _P : (k + 1) * _P, :])
            nc.sync.dma_start(w_s[k][:], w[k * _P : (k + 1) * _P, :])
        nc.vector.memset(ones[:], 1.0)
        make_identity(nc, ident[:])

        with tc.tile_pool(name="ps_esq", bufs=1, space="PSUM") as ps_esq:
            # pesq gets its own 2-bank pool because its final accumulation
            # overlaps main-loop chunk 0 (while ps_big holds the other 6)
            pesq = ps_esq.tile([_P, b_loc], f32, name="pesq", tag="pesq", bufs=1)

            def emit_sq_mul(j):
                sqe = scratch.tile([_P, 2, b_loc], bf16, name="sqe", tag="sqe")
                nc.vector.tensor_mul(sqe[:], et2_s[j][:], et2_s[j][:])
                return sqe

            def emit_pesq_mms(j, sqe):
                for k2 in range(2):
                    for nb in range(b_loc // _NT):
                        bs = slice(nb * _NT, (nb + 1) * _NT)
                        nc.tensor.matmul(
                            pesq[:, bs],
                            ones[:],
                            sqe[:, k2, bs],
                            start=(j == 0 and k2 == 0),
                            stop=(j == jd - 1 and k2 == 1),
                        )

            # x@W GEMM: 3 passes of one et-pair (2 m-blocks) each, k-outer
            # so compute starts as soon as the first k-tiles of xt/w land.
            # Pair j's square (DVE) is emitted right after its et2 evacs;
            # its pesq reduction matmuls one pass later (data long ready,
            # so the in-order PE queue never stalls on them).
            sqe_pend = None
            with tc.tile_pool(name="ps_pro", bufs=1, space="PSUM") as ps_pro:
                for pair in range(jd):
                    # rotate through 3 slots (6 banks; pesq holds the other 2)
                    pts = [
                        ps_pro.tile(
                            [_P, b_loc], f32, name=f"pro{pair}{mi}",
                            tag=f"pro{(2 * pair + mi) % 3}",
                        )
                        for mi in range(2)
                    ]
                    for k in range(kd):
                        for mi in range(2):
                            m = 2 * pair + mi
                            for nb in range(b_loc // _NT):
                                bs = slice(nb * _NT, (nb + 1) * _NT)
                                nc.tensor.matmul(
                                    pts[mi][:, bs],
                                    w_s[k][:, m * _P : (m + 1) * _P],
                                    xt_s[k][:, bs],
                                    start=(k == 0),
                                    stop=(k == kd - 1),
                                )
                    if sqe_pend is not None:
                        emit_pesq_mms(pair - 1, sqe_pend)
                    for mi in range(2):
                        m = 2 * pair + mi
                        nc.scalar.activation(
                            et2_s[pair][:, mi, :], pts[mi][:], AF.Copy, scale=2.0
                        )
                    sqe_pend = emit_sq_mul(pair)

            def emit_esq_tail():
                # final pair's reduction + negesq: emitted a few i-blocks
                # into main-loop chunk 0 so it overlaps the big GEMM
                emit_pesq_mms(jd - 1, sqe_pend)
                esq_rep = scratch.tile([_P, b_loc], f32, name="esq_rep", tag="esq_rep")
                nc.scalar.activation(esq_rep[:], pesq[:], AF.Copy)
                # transposes reuse pesq's PSUM slot (same tag ring)
                ptr = ps_esq.tile([_P, b_loc], f32, name="ptr", tag="pesq", bufs=1)
                for i in range(mb):
                    nc.tensor.transpose(
                        ptr[:, i * _P : (i + 1) * _P],
                        esq_rep[:, i * _P : (i + 1) * _P],
                        ident[:],
                    )
                for i in range(mb):
                    nc.scalar.activation(
                        negesq[:, i : i + 1], ptr[:, i * _P : i * _P + 1],
                        AF.Copy, scale=-0.25,
                    )

            big_pool = tc.alloc_tile_pool(name="ps_big", bufs=1, space="PSUM")

            def load_ct(n):
                csl = slice(n * _NW, (n + 1) * _NW)
                tiles = []
                for j in range(jd):
                    t = ct_pool.tile([_P, 2, _NW], e4, name=f"ct{j}", tag="ct")
                    nc.sync.dma_start(t[:], ct[j * _P : (j + 1) * _P, :, csl])
                    tiles.append(t)
                return tiles

            ct_cur = load_ct(0)
            # first two csq slices ride behind ct chunk 0 on the sync queue
            for h in range(2):
                hs = slice(h * csl_w, (h + 1) * csl_w)
                nc.sync.dma_start(csq_sb[:, hs], csqb[:, hs])
            ot_cur = {}

            def ot_slot(n, i):
                ot_cur[i] = out_pool.tile([_P, _NW], f16, name="ot", tag="ot")
                return ot_cur[i][:]

            def ot_store(n, i):
                nc.sync.dma_start(
                    out[i * _P : (i + 1) * _P, n * _NW : (n + 1) * _NW],
                    ot_cur[i][:],
                )

            def emit_evac(n, i, pb, csl):
                # t1 ACT must only be emitted once negesq's producer ACTs
                # are already in the (in-order) scalar queue, else deadlock
                t1 = t1_pool.tile([_P, _NW], f16, name="t1", tag="t1")
                nc.scalar.activation(
                    t1[:], pb[:], AF.Identity, bias=negesq[:, i : i + 1]
                )
                nc.vector.tensor_sub(ot_slot(n, i), t1[:], csq_sb[:, csl])
                ot_store(n, i)

            for n in range(npair):
                csl = slice(n * _NW, (n + 1) * _NW)
                ct_nxt = load_ct(n + 1) if n + 1 < npair else None
                if 1 <= n <= 6:
                    h = n + 1
                    hs = slice(h * csl_w, (h + 1) * csl_w)
                    nc.sync.dma_start(csq_sb[:, hs], csqb[:, hs])

                dve_tail = []
                for i in range(mb):
                    pb = big_pool.tile([_P, _NW], f32, name="big", tag="big", bufs=3)
                    for j in range(jd):
                        lhsT = et2_s[j][:, :, i * _P : (i + 1) * _P]
                        nc.tensor.matmul(
                            pb[:, 0:_NT], lhsT, ct_cur[j][:, :, 0:_NT],
                            start=(j == 0), stop=(j == jd - 1), perf_mode=DR,
                        )
                        nc.tensor.matmul(
                            pb[:, _NT:_NW], lhsT, ct_cur[j][:, :, _NT:_NW],
                            start=(j == 0), stop=(j == jd - 1), perf_mode=DR,
                        )
                    if n == 0 and i < 3:
                        # chunk 0, first tiles: DVE-only evac so their PSUM
                        # banks recycle without waiting on the ACT queue,
                        # which is busy with the esq tail (negesq producers)
                        tmp = t1_pool.tile([_P, _NW], f16, name="t1", tag="t1")
                        nc.vector.tensor_sub(tmp[:], pb[:], csq_sb[:, csl])
                        dve_tail.append((i, tmp))
                        if i == 2:
                            emit_esq_tail()
                            for ii, tmp_t in dve_tail:
                                nc.vector.tensor_scalar_add(
                                    ot_slot(n, ii), tmp_t[:], negesq[:, ii : ii + 1]
                                )
                                ot_store(n, ii)
                    else:
                        emit_evac(n, i, pb, csl)
                if ct_nxt is not None:
                    ct_cur = ct_nxt
            big_pool.release()


def build_nc(b_loc=_B_LOC, din=_DIN, d=_D, c=_C):
    import concourse.tile as tile
    from concourse import bacc, mybir

    nc = bacc.Bacc("TRN2", target_bir_lowering=False, debug=False)
    jd = d // (2 * _P)
    xt = nc.declare_dram_parameter("xt", [din, b_loc], mybir.dt.float16, isOutput=False)
    w = nc.declare_dram_parameter("w", [din, d], mybir.dt.float16, isOutput=False)
    ct = nc.declare_dram_parameter("ct", [jd * _P, 2, c], mybir.dt.float8e4, isOutput=False)
    csqb = nc.declare_dram_parameter("csqb", [_P, c], mybir.dt.float16, isOutput=False)
    out = nc.declare_dram_parameter("out", [b_loc, c], mybir.dt.float16, isOutput=True)
    with tile.TileContext(nc) as tc:
        emit_centroid_kernel(tc, xt.ap(), w.ap(), ct.ap(), csqb.ap(), out.ap(), b_loc, din, d, c)
    nc.compile()
    return nc


def _pack_pairs(a2d, dtype):
    k, f = a2d.shape
    j = k // (2 * _P)
    return np.ascontiguousarray(
        a2d.reshape(j, 2, _P, f).transpose(0, 2, 1, 3).reshape(j * _P, 2, f)
    ).astype(dtype)


def make_in_maps(x, W, centroids, b_loc=_B_LOC, n_cores=_NCORES):
    import ml_dtypes

    e4 = ml_dtypes.float8_e4m3

    x = np.asarray(x, dtype=np.float32)
    W = np.asarray(W, dtype=np.float32)
    centroids = np.asarray(centroids, dtype=np.float32)

    w_f16 = W.astype(np.float16)
    ct_p = _pack_pairs(np.ascontiguousarray(centroids.T), e4)
    xt_full = np.ascontiguousarray(x.T).astype(np.float16)

    # ||c||^2 from the quantized fp8 centroids (consistent with the
    # on-device cross GEMM so quantization errors cancel in the
    # perfect-square form), pre-broadcast across 128 partitions
    csq = (ct_p.astype(np.float32) ** 2).sum(axis=(0, 1))
    csqb = np.ascontiguousarray(
        np.broadcast_to(csq.astype(np.float16), (_P, csq.shape[0]))
    )

    maps = []
    for i in range(n_cores):
        xt_p = np.ascontiguousarray(xt_full[:, i * b_loc : (i + 1) * b_loc])
        maps.append({"xt": xt_p, "w": w_f16, "ct": ct_p, "csqb": csqb})
    return maps


_NC_CACHE = {}


def kernel(x, W, centroids):
    from concourse.bass_utils import run_bass_kernel_spmd

    if "nc" not in _NC_CACHE:
        _NC_CACHE["nc"] = build_nc()
    nc = _NC_CACHE["nc"]

    in_maps = make_in_maps(x, W, centroids)
    res = run_bass_kernel_spmd(nc, in_maps, list(range(_NCORES)))
    return np.concatenate(
        [res.results[i]["out"].astype(np.float32) for i in range(_NCORES)], axis=0
    )
